# revision 24
# baseline (speedup 1.0000x reference)
"""Depth2Normals Trainium2 Bass kernel (8 NeuronCores, one image per core).

Per image: 9x9 joint bilateral filter of depth (zeros = invalid), K rounds of
3x3 median hole-fill (K <= 7 determined host-side from pure hole-mask
morphology; the reference's remaining fills are exact identities), then
camera-space back-projection + central-difference cross-product normals.

Layout: the 480x640 image is row-padded to 648 cols and flattened to 311040
elements, split over 128 SBUF partitions (2430 each).  Each stencil input
lives in a per-partition halo buffer (partition p holds padded-flat range
[2430p - halo, 2430(p+1) + halo)), built by a single overlapped-window DMA
from a padded DRAM staging buffer, so every stencil tap is a pure free-dim
offset and all compute runs on full [128, 2430] tiles.

build_program(K, R) emits the whole pipeline R times back-to-back (identical
results; used to measure per-iteration hardware time by slope, cancelling the
axon-tunnel dispatch overhead).
"""

import math
import numpy as np

import concourse.bass as bass
import concourse.mybir as mybir

F32 = mybir.dt.float32
OP = mybir.AluOpType
AF = mybir.ActivationFunctionType

# ---------------- geometry ----------------
B, H, W = 8, 480, 640
WP = W + 8                 # padded row width
NPIX = H * WP              # 311040
NPART = 128
CH = NPIX // NPART         # 2430
R4 = 4
HALO4 = R4 * WP + R4       # 2596
XHW = CH + 2 * HALO4       # 7622
G4 = 2600
HALO1 = WP + 1             # 649
MFW = CH + 2 * HALO1       # 3728
G1 = 652
PSZ = G4 + NPIX + G4       # 316240
QSZ = G1 + NPIX + G1       # 312344

INV2SR = 50.0
INV2SD = 0.125
LNEG = -30.0

ARENA_COLS = 43520         # 170KB/partition
DRAIN_VS = True            # drain before each VE/ACT op (same-engine hazard model)


class Buf:
    def __init__(self, arena, name, start, n):
        self.arena, self.name, self.start, self.n = arena, name, start, n

    def v(self, a=0, n=None):
        if n is None:
            n = self.n - a
        return self.arena.t[:, self.start + a: self.start + a + n]

    def free(self):
        self.arena.free(self)


class Arena:
    def __init__(self, tensor, cols):
        self.t = tensor
        self.free_list = [(0, cols)]
        self.live = {}

    def alloc(self, name, n):
        na = (n + 15) & ~15
        for i, (s, ln) in enumerate(self.free_list):
            if ln >= na:
                self.live[name] = (s, na)
                if ln == na:
                    self.free_list.pop(i)
                else:
                    self.free_list[i] = (s + na, ln - na)
                return Buf(self, name, s, n)
        raise MemoryError(f"arena OOM for {name}:{n} free={self.free_list}")

    def free(self, buf):
        s, n = self.live.pop(buf.name)
        self.free_list.append((s, n))
        self.free_list.sort()
        out = []
        for seg in self.free_list:
            if out and out[-1][0] + out[-1][1] == seg[0]:
                out[-1] = (out[-1][0], out[-1][1] + seg[1])
            else:
                out.append(seg)
        self.free_list = out


class Prog:
    """Planned per-engine instruction streams with semaphore accounting.

    'v' = DVE, 's' = ACT, 'y' = SP (sync/HWDGE dma), 'p' = Pool (gpsimd).
    Compute engines execute in order: one semaphore each, +1 per instruction,
    waits are (engine, value).  DMAs complete out of order across queues, so
    each DMA *slot* (name) has its own semaphore (+16 per completion); a DMA
    issued as the n-th use of its slot yields token ('D', name, n) and
    waiting for it means sem >= 16n (deterministic because a slot's issues
    are ordered by SP program order and waited in order).
    """

    def __init__(self):
        self.items = {k: [] for k in 'vsyp'}
        self.cnt = {k: 0 for k in 'vsyp'}
        self.last_wait = {}
        self.dma_count = {}

    def op(self, eng, fn, inc=1, waits=()):
        real = []
        for wt in waits:
            if wt is None:
                continue
            if wt[0] == 'D':
                _, name, cnt = wt
                k = (eng, 'D', name)
                if self.last_wait.get(k, -1) >= cnt:
                    continue
                self.last_wait[k] = cnt
                real.append(wt)
                continue
            se, val = wt
            if val is None or val <= 0:
                continue
            if se == eng:
                continue
            k = (eng, se)
            if self.last_wait.get(k, -1) >= val:
                continue
            self.last_wait[k] = val
            real.append((se, val))
        self.items[eng].append((real, fn, inc))
        if not isinstance(inc, tuple):
            self.cnt[eng] += inc
        return self.cnt[eng]

    def dma(self, name, fn, waits=()):
        cnt = self.dma_count.get(name, 0) + 1
        self.dma_count[name] = cnt
        self.op('y', fn, inc=('D', name, cnt), waits=waits)
        return ('D', name, cnt)


def build_program(K, R=1):
    nc = bass.Bass("TRN2", target_bir_lowering=False, debug=False)
    AP = bass.AP

    depth = nc.dram_tensor("depth", [H, W], F32, kind="ExternalInput")
    intr8 = nc.dram_tensor("intr8", [NPART, 8], F32, kind="ExternalInput")
    ucol = nc.dram_tensor("ucol", [QSZ], F32, kind="ExternalInput")
    vrow = nc.dram_tensor("vrow", [QSZ], F32, kind="ExternalInput")
    out = nc.dram_tensor("out", [3, H, W], F32, kind="ExternalOutput")

    P = nc.dram_tensor("Pstage", [PSZ], F32)
    Q = nc.dram_tensor("Qstage", [QSZ], F32)
    O = nc.dram_tensor("Ostage", [3, NPIX], F32)

    arena_t = nc.alloc_sbuf_tensor("arena", [NPART, ARENA_COLS], F32)
    ar = Arena(arena_t, ARENA_COLS)
    smalls = nc.alloc_sbuf_tensor("smalls", [NPART, 16], F32)
    consts = nc.alloc_sbuf_tensor("consts", [NPART, 32], F32)
    # int8 predicate masks for CopyPredicated (BIR requires integer mask dtype)
    mask8 = nc.alloc_sbuf_tensor("mask8", [NPART, 4 * CH], mybir.dt.int8)

    pr = Prog()

    # per-tap Exp biases ln(w_s), pre-materialized as [128,1] const columns
    bias_col = {}
    for r2 in sorted({dy * dy + dx * dx for dy in range(-R4, R4 + 1)
                      for dx in range(-R4, R4 + 1)}):
        ws = float(np.float32(math.exp(-r2 * INV2SD)))
        lnws = float(np.float32(math.log(ws)))
        col = len(bias_col)
        bias_col[r2] = col
        pr.op('v', lambda e, col=col, v=lnws: e.memset(consts[:, col:col + 1], v))

    def bias_ap(r2):
        return consts[:, bias_col[r2]: bias_col[r2] + 1]

    ap_cx, ap_cy = smalls[:, 2:3], smalls[:, 3:4]
    ap_rfx, ap_rfy = smalls[:, 8:9], smalls[:, 9:10]
    ap_flag = smalls[:, 4:5]

    def V(fn, waits=()):
        return pr.op('v', fn, waits=waits)

    def TT(o, a, b, op, waits=()):
        return V(lambda e, o=o, a=a, b=b, op=op: e.tensor_tensor(o, a, b, op),
                 waits=waits)

    def emit_iter(prev_fouts):
        # ================= phase 0: staging prep =================
        zt = ar.alloc("zt", 3008)
        v_zt = pr.op('v', lambda e: e.memset(zt.v(), 0.0), waits=prev_fouts)

        pz_a, pz_b = 2470, PSZ - 2470 * NPART           # 316160 + 80
        d_z1 = pr.dma("z1", lambda e: e.dma_start(
            out=AP(P, 0, [[pz_a, NPART], [1, pz_a]]), in_=zt.v(0, pz_a)),
            waits=[('v', v_zt)])
        d_z2 = pr.dma("z2", lambda e: e.dma_start(
            out=AP(P, pz_a * NPART, [[pz_b, 1], [1, pz_b]]),
            in_=arena_t[0:1, zt.start: zt.start + pz_b]),
            waits=[('v', v_zt)])
        qz_a, qz_b = 2440, QSZ - 2440 * NPART           # 312320 + 24
        d_z3 = pr.dma("z3", lambda e: e.dma_start(
            out=AP(Q, 0, [[qz_a, NPART], [1, qz_a]]), in_=zt.v(0, qz_a)),
            waits=[('v', v_zt)])
        d_z4 = pr.dma("z4", lambda e: e.dma_start(
            out=AP(Q, qz_a * NPART, [[qz_b, 1], [1, qz_b]]),
            in_=arena_t[0:1, zt.start: zt.start + qz_b]),
            waits=[('v', v_zt)])
        d_depth = pr.dma("dep", lambda e: e.dma_start(
            out=AP(P, G4 + 4, [[WP, H], [1, W]]),
            in_=AP(depth, 0, [[W, H], [1, W]])),
            waits=[d_z2])
        xh = ar.alloc("xh", XHW)
        d_xh = pr.dma("xh", lambda e: e.dma_start(
            out=xh.v(), in_=AP(P, G4 - HALO4, [[CH, NPART], [1, XHW]])),
            waits=[d_depth])

        d_intr = pr.dma("intr", lambda e: e.dma_start(
            out=smalls[:, 0:8], in_=intr8[:, 0:8]), waits=[('v', v_zt)])
        V(lambda e: e.reciprocal(smalls[:, 8:10], smalls[:, 0:2]), waits=[d_intr])

        # pad-column mask (1 at the 640 real columns, 0 at the 8 pad columns)
        pm = ar.alloc("pm", CH)
        tpm = ar.alloc("tpm", CH)
        d_pm = pr.dma("pm", lambda e: e.dma_start(
            out=pm.v(), in_=AP(ucol, G1, [[CH, NPART], [1, CH]])),
            waits=[('v', v_zt)])
        V(lambda e: e.tensor_scalar(tpm.v(), pm.v(), 639.5, None, OP.is_le),
          waits=[d_pm])
        V(lambda e: e.scalar_tensor_tensor(
            pm.v(), pm.v(), -0.5, tpm.v(), OP.is_ge, OP.mult))
        tpm.free()

        d_all0 = [d_z1, d_z2, d_z3, d_z4, d_depth, d_xh, d_intr, d_pm]
        zt.free()

        # ============ phase 1: x' = (d>0 ? d : LNEG), in place on xh ========
        tmp_xh = ar.alloc("tmp_xh", XHW)
        V(lambda e: e.tensor_scalar(
            tmp_xh.v(), xh.v(), 0.0, LNEG, OP.is_le, OP.mult), waits=d_all0)
        V(lambda e: e.scalar_tensor_tensor(
            xh.v(), tmp_xh.v(), 0.0, xh.v(), OP.bypass, OP.add))
        tmp_xh.free()

        # ================= phase 2: bilateral =================
        num = ar.alloc("num", CH)
        den = ar.alloc("den", CH)
        wnb = ar.alloc("wnb", CH)
        sqt = ar.alloc("sqt", CH)
        diff = [ar.alloc("diff0", CH), ar.alloc("diff1", CH)]
        wbuf = [ar.alloc("w0", CH), ar.alloc("w1", CH)]

        taps = [(dy, dx) for dy in range(-R4, R4 + 1) for dx in range(-R4, R4 + 1)]
        OFF4 = HALO4
        xc = xh.v(OFF4, CH)

        def nbv(dy, dx):
            return xh.v(OFF4 + WP * dy + dx, CH)

        NT = len(taps)
        v_after_sub = [None] * NT
        s_after_exp = [None] * NT

        def emit_sub(k):
            dy, dx = taps[k]
            v_after_sub[k] = V(lambda e, k=k, dy=dy, dx=dx: e.tensor_tensor(
                diff[k % 2].v(), xc, nbv(dy, dx), OP.subtract))

        def emit_act(k):
            dy, dx = taps[k]
            bap = bias_ap(dx * dx + dy * dy)
            pr.op('s', lambda e, k=k: e.activation(
                sqt.v(), diff[k % 2].v(), AF.Square),
                waits=[('v', v_after_sub[k])])
            s_after_exp[k] = pr.op('s', lambda e, k=k, bap=bap: e.activation(
                wbuf[k % 2].v(), sqt.v(), AF.Exp, bias=bap, scale=-INV2SR))

        def emit_consume(k):
            dy, dx = taps[k]
            if k == 0:
                V(lambda e, k=k, dy=dy, dx=dx: e.tensor_tensor(
                    num.v(), wbuf[k % 2].v(), nbv(dy, dx), OP.mult),
                    waits=[('s', s_after_exp[k])])
                V(lambda e, k=k: e.tensor_copy(den.v(), wbuf[k % 2].v()))
            else:
                V(lambda e, k=k, dy=dy, dx=dx: e.tensor_tensor(
                    wnb.v(), wbuf[k % 2].v(), nbv(dy, dx), OP.mult),
                    waits=[('s', s_after_exp[k])])
                V(lambda e: e.tensor_tensor(num.v(), num.v(), wnb.v(), OP.add))
                V(lambda e, k=k: e.tensor_tensor(
                    den.v(), den.v(), wbuf[k % 2].v(), OP.add))

        emit_sub(0); emit_act(0)
        emit_sub(1); emit_act(1)
        for k in range(NT):
            emit_consume(k)
            if k + 2 < NT:
                emit_sub(k + 2)
                emit_act(k + 2)

        # ================= phase 3: filt = (num/den) * (x>0) =================
        filt, mp, rden = diff[0], diff[1], wnb
        V(lambda e: e.tensor_scalar(den.v(), den.v(), 1e-30, None, OP.max))
        V(lambda e: e.reciprocal(rden.v(), den.v()))
        V(lambda e: e.tensor_scalar(mp.v(), xc, 0.0, None, OP.is_gt))
        V(lambda e: e.tensor_tensor(filt.v(), num.v(), rden.v(), OP.mult))
        v_filt = V(lambda e: e.tensor_tensor(filt.v(), filt.v(), mp.v(), OP.mult))

        d_fout = pr.dma("fout", lambda e: e.dma_start(
            out=AP(Q, G1, [[CH, NPART], [1, CH]]), in_=filt.v()),
            waits=[('v', v_filt), d_z3, d_z4])

        xh.free()
        for bb in (num, den, sqt, wbuf[0], wbuf[1], diff[0], diff[1], wnb):
            bb.free()

        # ============ phase 4: K median-fill stages (all on VE) =============
        mf = [ar.alloc("mf_a", MFW), ar.alloc("mf_b", MFW)]
        d_win = pr.dma("win", lambda e: e.dma_start(
            out=mf[0].v(), in_=AP(Q, G1 - HALO1, [[CH, NPART], [1, MFW]])),
            waits=[d_fout])

        g = [ar.alloc(f"g{i}", CH) for i in range(13)]
        NOFF = [-WP - 1, -WP, -WP + 1, -1, 1, WP - 1, WP, WP + 1]
        OFF1 = HALO1

        for s in range(K):
            cur, nxt = mf[s % 2], mf[(s + 1) % 2]
            cc = cur.v(OFF1, CH)
            tv = [cur.v(OFF1 + o, CH) for o in NOFF]
            w_in = [d_win]

            cnt = g[12]
            V(lambda e, t=tv[0], o=cnt.v(): e.tensor_scalar(
                o, t, 0.0, None, OP.is_gt), waits=w_in)
            for i in range(1, 8):
                V(lambda e, t=tv[i], o=cnt.v(): e.scalar_tensor_tensor(
                    o, t, 0.0, o, OP.is_gt, OP.add))

            # sort4 of t0..t3 -> A = [g4, g0, g1, g7] ascending
            TT(g[0].v(), tv[0], tv[1], OP.min); TT(g[1].v(), tv[0], tv[1], OP.max)
            TT(g[2].v(), tv[2], tv[3], OP.min); TT(g[3].v(), tv[2], tv[3], OP.max)
            TT(g[4].v(), g[0].v(), g[2].v(), OP.min); TT(g[5].v(), g[0].v(), g[2].v(), OP.max)
            TT(g[6].v(), g[1].v(), g[3].v(), OP.min); TT(g[7].v(), g[1].v(), g[3].v(), OP.max)
            TT(g[0].v(), g[5].v(), g[6].v(), OP.min); TT(g[1].v(), g[5].v(), g[6].v(), OP.max)
            A = [g[4], g[0], g[1], g[7]]
            # sort4 of t4..t7 -> B = [g8, g2, g3, g11] ascending
            TT(g[2].v(), tv[4], tv[5], OP.min); TT(g[3].v(), tv[4], tv[5], OP.max)
            TT(g[5].v(), tv[6], tv[7], OP.min); TT(g[6].v(), tv[6], tv[7], OP.max)
            TT(g[8].v(), g[2].v(), g[5].v(), OP.min); TT(g[9].v(), g[2].v(), g[5].v(), OP.max)
            TT(g[10].v(), g[3].v(), g[6].v(), OP.min); TT(g[11].v(), g[3].v(), g[6].v(), OP.max)
            TT(g[2].v(), g[9].v(), g[10].v(), OP.min); TT(g[3].v(), g[9].v(), g[10].v(), OP.max)
            Bs = [g[8], g[2], g[3], g[11]]

            # bitonic split: c = min(A_i, B_{3-i}); d = max -> in-place over A
            cs = [g[5], g[6], g[9], g[10]]
            for i in range(4):
                TT(cs[i].v(), A[i].v(), Bs[3 - i].v(), OP.min)
                TT(A[i].v(), A[i].v(), Bs[3 - i].v(), OP.max)
            # s3 = max(c) -> g8
            TT(g[8].v(), g[5].v(), g[6].v(), OP.max)
            TT(g[2].v(), g[9].v(), g[10].v(), OP.max)
            TT(g[8].v(), g[8].v(), g[2].v(), OP.max)
            # bitonic sort of d = [g4, g0, g1, g7] -> s4..s7
            TT(g[5].v(), g[4].v(), g[1].v(), OP.min); TT(g[6].v(), g[4].v(), g[1].v(), OP.max)
            TT(g[9].v(), g[0].v(), g[7].v(), OP.min); TT(g[10].v(), g[0].v(), g[7].v(), OP.max)
            TT(g[0].v(), g[5].v(), g[9].v(), OP.min); TT(g[1].v(), g[5].v(), g[9].v(), OP.max)
            TT(g[4].v(), g[6].v(), g[10].v(), OP.min); TT(g[7].v(), g[6].v(), g[10].v(), OP.max)
            s3, s4, s5, s6, s7 = g[8], g[0], g[1], g[4], g[7]

            # count-threshold masks (int8 for CopyPredicated) + hole mask
            hb = g[9]
            m2 = mask8[:, 0 * CH:1 * CH]
            m4 = mask8[:, 1 * CH:2 * CH]
            m6 = mask8[:, 2 * CH:3 * CH]
            m8 = mask8[:, 3 * CH:4 * CH]
            V(lambda e, o=m2: e.tensor_scalar(o, cnt.v(), 1.5, None, OP.is_ge))
            V(lambda e, o=m4: e.tensor_scalar(o, cnt.v(), 3.5, None, OP.is_ge))
            V(lambda e, o=m6: e.tensor_scalar(o, cnt.v(), 5.5, None, OP.is_ge))
            V(lambda e, o=m8: e.tensor_scalar(o, cnt.v(), 7.5, None, OP.is_ge))
            V(lambda e, o=hb.v(), i=cc: e.tensor_scalar(o, i, 0.0, None, OP.is_le))

            # med = select by cnt  (g10)
            med = g[10]
            V(lambda e, o=med.v(), i=s7.v(): e.tensor_copy(o, i))
            V(lambda e, o=med.v(), m=m2, d=s6.v(): e.copy_predicated(o, m, d))
            V(lambda e, o=med.v(), m=m4, d=s5.v(): e.copy_predicated(o, m, d))
            V(lambda e, o=med.v(), m=m6, d=s4.v(): e.copy_predicated(o, m, d))
            V(lambda e, o=med.v(), m=m8, d=s3.v(): e.copy_predicated(o, m, d))

            # gate = (s7>0) * h * padmask ; out = cc + med*gate
            V(lambda e, o=g[0].v(), a=s7.v(), b=hb.v(): e.scalar_tensor_tensor(
                o, a, 0.0, b, OP.is_gt, OP.mult))
            TT(g[0].v(), g[0].v(), pm.v(), OP.mult)
            V(lambda e, o=g[1].v(), a=med.v(), b=g[0].v(): e.scalar_tensor_tensor(
                o, a, 0.0, b, OP.bypass, OP.mult))
            v_stage = TT(nxt.v(OFF1, CH), g[1].v(), cc, OP.add)

            d_out = pr.dma("mout", lambda e, nxt=nxt: e.dma_start(
                out=AP(Q, G1, [[CH, NPART], [1, CH]]), in_=nxt.v(OFF1, CH)),
                waits=[('v', v_stage)])
            d_win = pr.dma("win", lambda e, nxt=nxt: e.dma_start(
                out=nxt.v(), in_=AP(Q, G1 - HALO1, [[CH, NPART], [1, MFW]])),
                waits=[d_out])

        v_med_end = pr.cnt['v']
        for bb in g:
            bb.free()
        pm.free()

        # ================= phase 5: normals =================
        mf_fin = mf[K % 2]
        outd = mf[(K + 1) % 2]

        d0h = ar.alloc("d0h", MFW)
        ucb = ar.alloc("ucb", MFW)
        vrb = ar.alloc("vrb", MFW)
        d_d0h = pr.dma("d0h", lambda e: e.dma_start(
            out=d0h.v(), in_=AP(P, G4 - HALO1, [[CH, NPART], [1, MFW]])),
            waits=[('v', v_med_end)])
        d_uc = pr.dma("uc", lambda e: e.dma_start(
            out=ucb.v(), in_=AP(ucol, G1 - HALO1, [[CH, NPART], [1, MFW]])),
            waits=[('v', v_med_end)])
        d_vc = pr.dma("vc", lambda e: e.dma_start(
            out=vrb.v(), in_=AP(vrow, G1 - HALO1, [[CH, NPART], [1, MFW]])),
            waits=[('v', v_med_end)])

        # blend: outd = d0 + flag*(mf_fin - d0)  (full halo extent)
        V(lambda e: e.scalar_tensor_tensor(
            outd.v(), mf_fin.v(), 0.0, d0h.v(), OP.bypass, OP.subtract),
          waits=[d_win, d_d0h])
        V(lambda e: e.scalar_tensor_tensor(
            outd.v(), outd.v(), ap_flag, d0h.v(), OP.mult, OP.add))
        mf_fin.free()
        d0h.free()

        # valid mask, camera Z
        vv = ar.alloc("vv", MFW)
        V(lambda e: e.tensor_scalar(vv.v(), outd.v(), 6.0, None, OP.is_le))
        V(lambda e: e.scalar_tensor_tensor(
            vv.v(), outd.v(), 0.1, vv.v(), OP.is_ge, OP.mult))
        Z = ar.alloc("Z", MFW)
        V(lambda e: e.tensor_tensor(Z.v(), outd.v(), vv.v(), OP.mult))
        vv.free()
        outd.free()

        OFF1 = HALO1
        gz = ar.alloc("gz", CH)
        gz2 = ar.alloc("gz2", CH)
        TT(gz.v(), Z.v(OFF1 + 1, CH), Z.v(OFF1 - 1, CH), OP.min)
        TT(gz2.v(), Z.v(OFF1 + WP, CH), Z.v(OFF1 - WP, CH), OP.min)
        TT(gz.v(), gz.v(), gz2.v(), OP.min)
        TT(gz.v(), gz.v(), Z.v(OFF1, CH), OP.min)
        gz2.free()

        # A = (u-cx)/fx in-place on ucb; B = (v-cy)/fy; X = A*Z; Y = B*Z
        V(lambda e: e.tensor_scalar(
            ucb.v(), ucb.v(), ap_cx, ap_rfx, OP.subtract, OP.mult), waits=[d_uc])
        V(lambda e: e.tensor_scalar(
            vrb.v(), vrb.v(), ap_cy, ap_rfy, OP.subtract, OP.mult), waits=[d_vc])
        X = ar.alloc("X", MFW)
        Y = ar.alloc("Y", MFW)
        V(lambda e: e.tensor_tensor(X.v(), ucb.v(), Z.v(), OP.mult))
        V(lambda e: e.tensor_tensor(Y.v(), vrb.v(), Z.v(), OP.mult))
        ucb.free()
        vrb.free()

        # central differences (output extent)
        dxX = ar.alloc("dxX", CH); dyX = ar.alloc("dyX", CH)
        dxY = ar.alloc("dxY", CH); dyY = ar.alloc("dyY", CH)
        dxZ = ar.alloc("dxZ", CH); dyZ = ar.alloc("dyZ", CH)
        for (db, src) in ((dxX, X), (dxY, Y), (dxZ, Z)):
            TT(db.v(), src.v(OFF1 + 1, CH), src.v(OFF1 - 1, CH), OP.subtract)
        for (db, src) in ((dyX, X), (dyY, Y), (dyZ, Z)):
            TT(db.v(), src.v(OFF1 + WP, CH), src.v(OFF1 - WP, CH), OP.subtract)
        X.free(); Y.free(); Z.free()

        # cross product n = dy_vec x dx_vec
        m1 = ar.alloc("m1", CH); m2b = ar.alloc("m2b", CH)
        nx = ar.alloc("nx", CH); ny = ar.alloc("ny", CH); nz = ar.alloc("nz", CH)
        TT(m1.v(), dyY.v(), dxZ.v(), OP.mult)
        TT(m2b.v(), dyZ.v(), dxY.v(), OP.mult)
        TT(nx.v(), m1.v(), m2b.v(), OP.subtract)
        TT(m1.v(), dyZ.v(), dxX.v(), OP.mult)
        TT(m2b.v(), dyX.v(), dxZ.v(), OP.mult)
        TT(ny.v(), m1.v(), m2b.v(), OP.subtract)
        TT(m1.v(), dyX.v(), dxY.v(), OP.mult)
        TT(m2b.v(), dyY.v(), dxX.v(), OP.mult)
        v_nz = TT(nz.v(), m1.v(), m2b.v(), OP.subtract)
        for bb in (dxX, dyX, dxY, dyY, dxZ, dyZ, m1, m2b):
            bb.free()

        # nn2 = nx^2+ny^2+nz^2 (squares on ACT), nn = sqrt, rinv = 1/nn
        sq1 = ar.alloc("sq1", CH); sq2 = ar.alloc("sq2", CH); sq3 = ar.alloc("sq3", CH)
        pr.op('s', lambda e: e.activation(sq1.v(), nx.v(), AF.Square),
              waits=[('v', v_nz)])
        pr.op('s', lambda e: e.activation(sq2.v(), ny.v(), AF.Square))
        s_q3 = pr.op('s', lambda e: e.activation(sq3.v(), nz.v(), AF.Square))
        nn2 = sq1
        TT(nn2.v(), sq1.v(), sq2.v(), OP.add, waits=[('s', s_q3)])
        TT(nn2.v(), nn2.v(), sq3.v(), OP.add)
        qb = sq2
        V(lambda e: e.tensor_scalar(qb.v(), nn2.v(), 1e-16, None, OP.is_gt))
        v_nn2 = V(lambda e: e.tensor_scalar(nn2.v(), nn2.v(), 1e-30, None, OP.max))
        nn = sq3
        s_nn = pr.op('s', lambda e: e.activation(nn.v(), nn2.v(), AF.Sqrt),
                     waits=[('v', v_nn2)])
        rinv = nn2
        V(lambda e: e.reciprocal(rinv.v(), nn.v()), waits=[('s', s_nn)])

        # gate = (zmin>0)*q ; rg = rinv*gate ; n_out = n * rg
        V(lambda e: e.scalar_tensor_tensor(
            gz.v(), gz.v(), 0.0, qb.v(), OP.is_gt, OP.mult))
        TT(gz.v(), gz.v(), rinv.v(), OP.mult)
        v_ox = TT(nx.v(), nx.v(), gz.v(), OP.mult)
        v_oy = TT(ny.v(), ny.v(), gz.v(), OP.mult)
        v_oz = TT(nz.v(), nz.v(), gz.v(), OP.mult)

        # ================= phase 6: outputs =================
        fouts = []
        for i, (buf, vdone) in enumerate(((nx, v_ox), (ny, v_oy), (nz, v_oz))):
            d_o = pr.dma(f"o{i}", lambda e, i=i, buf=buf: e.dma_start(
                out=AP(O, i * NPIX, [[CH, NPART], [1, CH]]), in_=buf.v()),
                waits=[('v', vdone)])
            fouts.append(pr.dma(f"f{i}", lambda e, i=i: e.dma_start(
                out=AP(out, i * H * W, [[W, H], [1, W]]),
                in_=AP(O, i * NPIX + 4, [[WP, H], [1, W]])),
                waits=[d_o]))
        for bb in (nx, ny, nz, sq1, sq2, sq3, gz):
            bb.free()
        return fouts

    fouts = []
    for r in range(R):
        fouts = emit_iter(fouts)

    # ================= emit =================
    import contextlib
    with contextlib.ExitStack() as stack:
        block = stack.enter_context(nc.Block())
        sems = {k: stack.enter_context(nc.semaphore(f"sem_{k}")) for k in 'vsyp'}
        dsems = {name: stack.enter_context(nc.semaphore(f"dma_{name}"))
                 for name in pr.dma_count}

        def replay(key):
            def run(eng):
                own = sems[key]
                first = True
                for waits, fn, inc in pr.items[key]:
                    for wt in waits:
                        if wt[0] == 'D':
                            eng.wait_ge(dsems[wt[1]], 16 * wt[2])
                        else:
                            eng.wait_ge(sems[wt[0]], wt[1])
                    if DRAIN_VS and key in 'vs' and not first:
                        eng.drain()
                    first = False
                    inst = fn(eng)
                    if isinstance(inc, tuple):
                        inst.then_inc(dsems[inc[1]], 16)
                    else:
                        inst.then_inc(own, inc)
            return run

        block.vector(replay('v'))
        block.scalar(replay('s'))
        block.sync(replay('y'))
        block.gpsimd(replay('p'))

    return nc


# ---------------- host side ----------------

_PROG_CACHE = {}
LAST_RESULTS = None


def _fill_stages_needed(holes):
    """Exact mask-only simulation of the reference's (up to 7) median fills.

    A zero pixel is filled iff it has a nonzero 8-neighbor; filled pixels stay
    nonzero forever, and nonzero pixels never change.  Returns the number of
    fill stages after which the zero-mask is stable (further fills are exact
    identities on values too, since stable zeros have all-zero neighborhoods).
    """
    z = holes.copy()
    Kmax = 0
    for step in range(7):
        nzp = np.pad(~z, ((0, 0), (1, 1), (1, 1)))
        any_nb = np.zeros_like(z)
        for dy in (0, 1, 2):
            for dx in (0, 1, 2):
                if dy == 1 and dx == 1:
                    continue
                any_nb |= nzp[:, dy:dy + z.shape[1], dx:dx + z.shape[2]]
        fill = z & any_nb
        if not fill.any():
            break
        z &= ~fill
        Kmax = step + 1
    return Kmax


def _const_maps():
    gidx = np.arange(NPIX, dtype=np.int64)
    u = (gidx % WP - 4).astype(np.float32)
    v = (gidx // WP).astype(np.float32)
    uc = np.zeros(QSZ, np.float32)
    vr = np.zeros(QSZ, np.float32)
    uc[G1:G1 + NPIX] = u
    vr[G1:G1 + NPIX] = v
    return uc, vr


def kernel(depth, intrinsic_params, _trace=False):
    global LAST_RESULTS
    from concourse.bass_utils import run_bass_kernel_spmd

    depth = np.asarray(depth, np.float32)
    intr = np.asarray(intrinsic_params, np.float32)
    d0 = depth[:, 0]                       # [B,H,W]
    holes = d0 == 0.0
    flags = holes.any(axis=(1, 2)).astype(np.float32)
    K = _fill_stages_needed(holes) if flags.any() else 0

    if K not in _PROG_CACHE:
        _PROG_CACHE[K] = build_program(K)
    nc = _PROG_CACHE[K]

    uc, vr = _const_maps()
    in_maps = []
    for b in range(B):
        i8 = np.zeros(8, np.float32)
        i8[0:4] = intr[b]
        i8[4] = flags[b]
        in_maps.append({
            "depth": np.ascontiguousarray(d0[b]),
            "intr8": np.tile(i8, (NPART, 1)),
            "ucol": uc,
            "vrow": vr,
        })
    res = run_bass_kernel_spmd(nc, in_maps, core_ids=list(range(B)), trace=_trace)
    LAST_RESULTS = res
    return np.stack([r["out"] for r in res.results]).astype(np.float32)
